# revision 13
# baseline (speedup 1.0000x reference)
"""LoRA QKV linear for TRN2, 8-way tensor-parallel on the output dim.

Math: q/k/v = x @ (W^T + s*A@B) + bias  (LoRA folded into the weight on host,
exact algebraic identity).  Each of the 8 cores owns a 768-wide column slice
of the concatenated [Q(4096) | K(1024) | V(1024)] output: 512 Q cols + 128 K
cols + 128 V cols.  x is pre-transposed on the host to xT [H, M] so both
matmul operands arrive with the contraction dim (H) on SBUF partitions with
contiguous, DMA-friendly access patterns (no on-chip transposes).

Matmuls run in float32r (TF32-like, fp22 multiply, fp32 accumulate): full
1 cycle/row PE rate when the moving free dim >= 256, i.e. 4x faster than
strict fp32 with ~1e-4 relative error.

Built on bacc.Bacc (not bass.Bass): its finalize() runs
generate_event_semaphores / move_matmul_waits_to_ldweights, which legalize
Tile's multi-wait instructions for this walrus (1 sync wait per instruction).
"""

import numpy as np

import concourse.bass as bass
import concourse.mybir as mybir
import concourse.tile as tile
from concourse import bacc, bass_utils

B, S, H = 4, 2048, 4096
Q_DIM, KV_DIM, R = 4096, 1024, 16
SCALING = 32.0 / 16.0
N_CORES = 8
QC = Q_DIM // N_CORES       # 512 Q cols per core
KC = KV_DIM // N_CORES      # 128 K (and V) cols per core
N_OUT = QC + 2 * KC         # 768 output cols per core
M = B * S                   # 8192 tokens
P = 128
K_TILES = H // P            # 32
M_TILES = M // P            # 64

# test.py reads these after calling kernel()
last_results = None

_cached_nc = None


def _build():
    nc = bacc.Bacc(
        "TRN2", target_bir_lowering=False, debug=False, num_devices=N_CORES
    )
    xT = nc.dram_tensor("xT", [H, M], mybir.dt.float32r, kind="ExternalInput").ap()
    w = nc.dram_tensor("w", [H, N_OUT], mybir.dt.float32r, kind="ExternalInput").ap()
    bias = nc.dram_tensor("bias", [P, N_OUT], mybir.dt.float32, kind="ExternalInput").ap()
    out = nc.dram_tensor("out", [M, N_OUT], mybir.dt.float32, kind="ExternalOutput").ap()

    xT3 = xT.rearrange("(ko p) m -> p ko m", p=P)
    w3 = w.rearrange("(ko p) n -> p ko n", p=P)

    with tile.TileContext(nc) as tc:
        with (
            tc.tile_pool(name="wpool", bufs=1) as wpool,
            tc.tile_pool(name="xpool", bufs=3) as xpool,
            tc.tile_pool(name="opool", bufs=3) as opool,
            tc.tile_pool(name="psum", bufs=8, space="PSUM") as psum_pool,
        ):
            w_sb = wpool.tile([P, K_TILES, N_OUT], mybir.dt.float32r)
            nc.sync.dma_start(w_sb, w3)
            b_sb = wpool.tile([P, N_OUT], mybir.dt.float32)
            nc.sync.dma_start(b_sb, bias)

            for mt in range(M_TILES):
                x_sb = xpool.tile([P, K_TILES, P], mybir.dt.float32r, tag="x")
                nc.sync.dma_start(x_sb, xT3[:, :, mt * P : (mt + 1) * P])
                o_sb = opool.tile([P, N_OUT], mybir.dt.float32, tag="o")
                for n0, n1 in ((0, 512), (512, N_OUT)):
                    ps = psum_pool.tile([P, 512], mybir.dt.float32, tag="ps")
                    pw = ps[:, : n1 - n0]
                    for ko in range(K_TILES):
                        nc.tensor.matmul(
                            pw,
                            x_sb[:, ko, :],
                            w_sb[:, ko, n0:n1],
                            start=(ko == 0),
                            stop=(ko == K_TILES - 1),
                        )
                    nc.vector.tensor_add(o_sb[:, n0:n1], pw, b_sb[:, n0:n1])
                nc.sync.dma_start(out[mt * P : (mt + 1) * P, :], o_sb)

    nc.finalize()
    return nc


def kernel(x, Wq, Wk, Wv, bq, bk, bv, Aq, Bq, Ak, Bk, Av, Bv):
    global last_results, _cached_nc

    x = np.asarray(x, dtype=np.float32)
    xT = np.ascontiguousarray(x.reshape(M, H).T)  # [H, M]

    # Fold LoRA into the base weight (in float64 for a clean merge), build
    # per-core column slices of the transposed effective weight.
    in_maps = []
    for c in range(N_CORES):
        wq = np.asarray(Wq)[c * QC : (c + 1) * QC].astype(np.float64)  # [512, H]
        wk = np.asarray(Wk)[c * KC : (c + 1) * KC].astype(np.float64)  # [128, H]
        wv = np.asarray(Wv)[c * KC : (c + 1) * KC].astype(np.float64)
        w_eff = np.concatenate([wq, wk, wv], axis=0).T  # [H, 768]
        w_eff[:, :QC] += SCALING * (
            np.asarray(Aq, np.float64) @ np.asarray(Bq, np.float64)[:, c * QC : (c + 1) * QC]
        )
        w_eff[:, QC : QC + KC] += SCALING * (
            np.asarray(Ak, np.float64) @ np.asarray(Bk, np.float64)[:, c * KC : (c + 1) * KC]
        )
        w_eff[:, QC + KC :] += SCALING * (
            np.asarray(Av, np.float64) @ np.asarray(Bv, np.float64)[:, c * KC : (c + 1) * KC]
        )
        b_c = np.concatenate(
            [
                np.asarray(bq)[c * QC : (c + 1) * QC],
                np.asarray(bk)[c * KC : (c + 1) * KC],
                np.asarray(bv)[c * KC : (c + 1) * KC],
            ]
        ).astype(np.float32)
        in_maps.append(
            {
                "xT": xT,
                "w": np.ascontiguousarray(w_eff, dtype=np.float32),
                "bias": np.ascontiguousarray(
                    np.broadcast_to(b_c, (P, N_OUT)), dtype=np.float32
                ),
            }
        )

    if _cached_nc is None:
        _cached_nc = _build()

    res = bass_utils.run_bass_kernel_spmd(
        _cached_nc, in_maps, core_ids=list(range(N_CORES))
    )
    last_results = res

    outs = [r["out"] for r in res.results]  # each [M, 768]
    q = np.concatenate([o[:, :QC] for o in outs], axis=1).reshape(B, S, Q_DIM)
    k = np.concatenate([o[:, QC : QC + KC] for o in outs], axis=1).reshape(B, S, KV_DIM)
    v = np.concatenate([o[:, QC + KC :] for o in outs], axis=1).reshape(B, S, KV_DIM)
    return q, k, v
